# revision 39
# baseline (speedup 1.0000x reference)
"""LIF spiking-neuron (BaseNeuron) forward kernel for Trainium2.

Reference semantics (per element, over T=16 timesteps):
    decay_s = sigmoid(decay)                     # scalar
    mem_t   = mem_{t-1} * decay_s * (1 - spike_{t-1}) + x_t
    spike_t = (mem_t > 0.5)
    out     = spikes (0.0/1.0 fp32), clipped to [0,1] (no-op)

Sharding: pure data parallel over batch B=16 across 8 NeuronCores
(2 batch rows per core). Per core the shard [2, 64, 16, 64, 64] is
viewed as [128 rows=(b,c), 16*4096 cols=(t,h,w)].

Per-core schedule (the serial DVE chain, 30 STT ops ~134us, is the
hard wall: GPSIMD can't run STT and shares DVE's SBUF port, ACT can't
do tensor-tensor, custom fused DVE ops don't compile in this build):
  DVE:  M_t = R_{t-1}*d + x_t (STT), R_t = (M<=0.5)*M (STT) - serial chain
  ACT:  S_t = Sign(Relu(M_t - 0.5)) -> bf16 (spikes), PSUM->SBUF copies
  PE:   bit-pack 24 spikes -> 1 fp32 int via matmul with powers-of-2
        weights, into PSUM rows 32*(t%4)..+6
  DMA:  one HWDGE ring tops out ~110-190 GB/s, so input x is spread
        over all three rings (sync HWDGE / gpsimd SWDGE / scalar
        HWDGE), 1-timestep chunks with 3-deep prefetch, first steps
        split across rings to shorten the ramp; packed rows drain on
        the gpsimd ring mid-kernel; t=12..15 skip the pack and stream
        raw bf16 spikes on idle rings so nothing heavy sits in the
        tail after the last chain op.

Output compression: 24 spikes/partition-group packed into one exact
fp32 integer (values < 2^24), decoded on host; cuts output HBM traffic
32 MiB -> ~3 MiB per core, keeping the kernel chain-bound rather than
DMA-bound.  Everything is bit-exact vs the fp32 reference (rel err 0.0).
"""

import os
import sys

sys.path.insert(0, "/opt/trn_rl_repo")

import numpy as np

_N_CORES = 8
_B, _C, _T, _H, _W = 16, 64, 16, 64, 64
_BPC = _B // _N_CORES            # batch rows per core = 2
_ROWS = _BPC * _C                # 128 partitions
_FD = _H * _W                    # 4096 elements per t per partition
_THRESH = 0.5
_NG = 6                          # bit-pack groups (24+24+24+24+24+8 = 128)

# filled by kernel() when tracing is enabled via BASS_TRACE=1
last_results = None

_cache = {}


def _build(decay_s: float):
    import concourse.bass as bass
    import concourse.tile as tile
    from concourse import mybir
    from contextlib import ExitStack

    f32 = mybir.dt.float32
    bf16 = mybir.dt.bfloat16
    Alu = mybir.AluOpType
    Act = mybir.ActivationFunctionType

    nc = bass.Bass("TRN2", target_bir_lowering=False, debug=False)
    x_d = nc.dram_tensor("x", [_ROWS, _T * _FD], f32, kind="ExternalInput").ap()
    w_d = nc.dram_tensor("w", [_ROWS, _NG], bf16, kind="ExternalInput").ap()
    op_d = nc.dram_tensor(
        "outp", [_NG * (_T - 4), _FD], f32, kind="ExternalOutput"
    ).ap()
    f8 = mybir.dt.float8e4
    # t=12/13 drain mid-kernel (bf16); t=14/15 gate the kernel end, so they
    # go out as fp8 (exact for 0/1 spike values) to halve the tail transfer
    os_d = [
        nc.dram_tensor(
            f"outs{t}", [_ROWS, _FD], bf16 if t < _T - 2 else f8,
            kind="ExternalOutput",
        ).ap()
        for t in range(_T - 4, _T)
    ]

    with tile.TileContext(nc) as tc, ExitStack() as ctx:
        cp = ctx.enter_context(tc.tile_pool(name="cst", bufs=1))
        xp = ctx.enter_context(tc.tile_pool(name="xin", bufs=4))
        mp = ctx.enter_context(tc.tile_pool(name="mem", bufs=2))
        rp = ctx.enter_context(tc.tile_pool(name="ret", bufs=2))
        sp = ctx.enter_context(tc.tile_pool(name="spk", bufs=3))
        s15p = ctx.enter_context(tc.tile_pool(name="spk15", bufs=1))
        op_ = ctx.enter_context(tc.tile_pool(name="stg", bufs=1))
        pp = ctx.enter_context(tc.tile_pool(name="ps", bufs=1, space="PSUM"))

        W = cp.tile([_ROWS, _NG], bf16)
        nc.scalar.dma_start(W[:], w_d[:])
        nthr = cp.tile([_ROWS, 1], f32)
        nc.vector.memset(nthr[:], -_THRESH)

        P = pp.tile([_ROWS, _FD], f32)

        # one X tile per timestep; each queue alone only sustains ~110 GB/s
        # under full-chip load, so input is spread over all three DMA rings
        # (sync HWDGE, gpsimd SWDGE, scalar HWDGE).  The first two timesteps
        # are themselves split across two rings to shorten the ramp.
        xtiles = []

        def load_x(t):
            Xt = xp.tile([_ROWS, _FD], f32, name="Xt")
            xtiles.append(Xt)
            if t < 4:
                # ramp: split the first timesteps across rings so the serial
                # chain starts as early as possible; t=1 also uses the (slow
                # but idle-at-start) scalar ring
                qs = (
                    (nc.sync, nc.gpsimd)
                    if t != 1
                    else (nc.scalar, nc.sync, nc.gpsimd)
                )
                n = len(qs)
                cut = [_FD * i // n for i in range(n + 1)]
                for q, lo, hi in zip(qs, cut[:-1], cut[1:]):
                    q.dma_start(
                        Xt[:, lo:hi], x_d[:, t * _FD + lo : t * _FD + hi]
                    )
            else:
                q = (nc.sync, nc.gpsimd, nc.scalar)[(t - 4) % 3]
                q.dma_start(Xt[:], x_d[:, bass.ts(t, _FD)])

        for t in range(3):
            load_x(t)

        R_prev = None
        for t in range(_T):
            if t + 3 < _T:
                load_x(t + 3)
            xt = xtiles[t][:]

            if t == 0:
                M = xt
            else:
                Mt = mp.tile([_ROWS, _FD], f32)
                nc.vector.scalar_tensor_tensor(
                    Mt[:], R_prev, decay_s, xt, Alu.mult, Alu.add
                )
                M = Mt[:]

            if t < _T - 1:
                Rt = rp.tile([_ROWS, _FD], f32)
                nc.vector.scalar_tensor_tensor(
                    Rt[:], M, _THRESH, M, Alu.is_le, Alu.mult
                )
                R_prev = Rt[:]

            # spikes S_t = (M_t > 0.5) as exact 0/1 bf16
            half = _FD // 2
            if t == _T - 2:
                # ACT owns all of S14 (keeping the DVE queue clear so M15 is
                # never delayed by a hoisted spike op).  Relu intermediate
                # stays bf16 (near-threshold values would underflow in fp8);
                # the exact 0/1 Sign output goes to fp8 and out in halves.
                Sb = sp.tile([_ROWS, _FD], bf16)
                S14 = s15p.tile([_ROWS, _FD], f8, name="S14f8")
                nc.scalar.activation(Sb[:], M, Act.Relu, bias=nthr[:], scale=1.0)
                nc.scalar.activation(S14[:], Sb[:], Act.Sign)
                nc.sync.dma_start(os_d[2][:, 0:half], S14[:, 0:half])
                nc.gpsimd.dma_start(os_d[2][:, half:_FD], S14[:, half:_FD])
                continue
            if t == _T - 1:
                # tail: DVE is idle after the last chain op
                S15 = s15p.tile([_ROWS, _FD], f8, name="S15f8")
                nc.vector.tensor_scalar(S15[:], M, _THRESH, None, Alu.is_gt)
                nc.sync.dma_start(os_d[3][:, 0:half], S15[:, 0:half])
                nc.gpsimd.dma_start(os_d[3][:, half:_FD], S15[:, half:_FD])
                break

            S = sp.tile([_ROWS, _FD], bf16)
            nc.scalar.activation(S[:], M, Act.Relu, bias=nthr[:], scale=1.0)
            nc.scalar.activation(S[:], S[:], Act.Sign)

            if t >= _T - 4:
                # t=12..15 skip the pack: raw bf16 spikes stream straight out
                # on by-then-idle rings (the copy+pack path would finish the
                # last batch ~12us later at the tail).  ACT only dispatches
                # after its final spike so spikes are never delayed.
                qa, qb = nc.sync, nc.gpsimd
                qa.dma_start(os_d[t - 12][:, 0:half], S[:, 0:half])
                qb.dma_start(os_d[t - 12][:, half:_FD], S[:, half:_FD])
                continue

            # bit-pack: psum[32k+j, f] = sum_i 2^i * S[24j+i, f]
            k = 32 * (t % 4)
            for c in range(8):
                nc.tensor.matmul(
                    P[k : k + _NG, bass.ts(c, 512)],
                    W[:],
                    S[:, bass.ts(c, 512)],
                    start=True,
                    stop=True,
                    tile_position=(0, k),
                )

            if t % 4 == 3:
                # drain finished timesteps: PSUM -> SBUF staging (ACT), then
                # out on the gpsimd SWDGE ring (keeps the ACT engine
                # compute-only and the sync ring free for input)
                t0b = t - (t % 4)
                O = op_.tile([_ROWS, _FD], f32)
                nc.scalar.activation(O[0:102, :], P[0:102, :], Act.Copy)
                for tt in range(t0b, t + 1):
                    kk = 32 * (tt % 4)
                    nc.gpsimd.dma_start(
                        op_d[_NG * tt : _NG * (tt + 1), :], O[kk : kk + _NG, :]
                    )

    _prune_redundant_waits(nc)
    return nc


def _prune_redundant_waits(nc) -> int:
    """Drop semaphore waits that are transitively implied by the instruction's
    other waits / proc program order.

    Tile's wait assignment is per-proc minimal but not transitively minimal
    (documented), and this walrus build rejects DMACopy instructions carrying
    more than one sync-wait command.  Reasoning model: every instruction
    belongs to a serial proc (engine, or DMA issue queue).  A wait (s >= v)
    observed by instruction i guarantees completion of every update event e of
    s for which max-possible-sum-excluding-e < v, where the feasible completed
    sets are per-proc prefixes of s's updaters, and events issued on i's own
    proc at/after i are excluded.  Guarantees propagate through event
    completion closures.
    """
    from concourse import mybir

    insts = []
    inst_loc = []  # (block, local index) per instruction
    for blk in nc.m.functions[0].blocks:
        for li, ins in enumerate(blk.instructions):
            insts.append(ins)
            inst_loc.append((blk, li))

    def proc_of(ins):
        q = getattr(ins, "queue", None)
        if q:
            return ("q", q)
        return ("e", str(ins.engine))

    def waits_of(ins):
        si = ins.sync_info
        if si is None:
            return []
        return list(si.on_wait or [])

    def updates_of(ins):
        si = ins.sync_info
        if si is None:
            return []
        return list(si.on_update or [])

    def semkey(ref):
        return (str(ref.sync_type), ref.id)

    def add_value(u):
        """positive increment if u is a plain additive update, else None"""
        if u.update_mode in ("sem-add-imm", "sem-inc") and (
            u.update_value is not None and u.update_value > 0
        ):
            return u.update_value
        return None

    # pass 1: find the first non-additive update per sem ("dirty point")
    dirty_from = {}
    for idx, ins in enumerate(insts):
        for u in updates_of(ins):
            if add_value(u) is None:
                dirty_from.setdefault(semkey(u), idx)

    # forward pass
    def merge(dst, src):
        for k, v in src.items():
            if dst.get(k, -1) < v:
                dst[k] = v

    proc_g = {}          # proc -> guarantee dict {semkey: value}
    events = {}          # semkey -> list of (idx, proc, inc, cum_after, guarantees)
    n_pruned = 0
    splits = []          # (flat idx, instruction, waits to move out)

    for idx, ins in enumerate(insts):
        p = proc_of(ins)
        base = dict(proc_g.get(p, {}))

        def resolve(w):
            """guarantees implied by wait w at instruction idx on proc p"""
            k = semkey(w)
            out = {}
            if w.wait_mode != "sem-ge-imm" or w.wait_value is None:
                return out
            v = w.wait_value
            out[k] = v
            if k in dirty_from and dirty_from[k] <= idx:
                return out
            evs = [e for e in events.get(k, []) if not (e[1] == p and e[0] >= idx)]
            total = sum(e[2] for e in evs)
            proc_total = {}
            for e in evs:
                proc_total[e[1]] = proc_total.get(e[1], 0) + e[2]
            # event e is guaranteed-complete iff even with every other proc
            # fully done and e's own proc stopped just before e, v can't be
            # reached: (total - proc_total[e.proc] + prefix_before_e) < v
            prefix = {}
            for e in evs:
                pre = prefix.get(e[1], 0)
                if total - proc_total[e[1]] + pre < v:
                    merge(out, e[4])
                prefix[e[1]] = pre + e[2]
            return out

        ws = waits_of(ins)
        if len(ws) > 1:
            # try to prune redundant waits
            keep = list(ws)
            changed = True
            while changed and len(keep) > 1:
                changed = False
                for j, w in enumerate(keep):
                    if w.wait_mode != "sem-ge-imm" or w.wait_value is None:
                        continue
                    g = dict(base)
                    for k2, w2 in enumerate(keep):
                        if k2 != j:
                            merge(g, resolve(w2))
                    if g.get(semkey(w), -1) >= w.wait_value:
                        keep.pop(j)
                        n_pruned += 1
                        changed = True
                        break
            if len(keep) != len(ws):
                ins.sync_info.on_wait = keep
                ws = keep
            if len(keep) > 1:
                # this walrus build accepts at most one sync-wait command per
                # instruction: move the extras onto standalone EventSemaphore
                # instructions placed just before it on the same engine
                splits.append((idx, ins, keep[:-1]))
                ins.sync_info.on_wait = keep[-1:]

        # start guarantees (use the original semantics: all kept waits hold)
        g_start = dict(base)
        for w in ws:
            merge(g_start, resolve(w))

        for u in updates_of(ins):
            k = semkey(u)
            if k in dirty_from and dirty_from[k] <= idx:
                continue
            inc = add_value(u)
            if inc is not None:
                evs = events.setdefault(k, [])
                cum = (evs[-1][3] if evs else 0) + inc
                ev_g = dict(g_start)
                ev_g[k] = cum
                evs.append((idx, p, inc, cum, ev_g))

        # Successors on this proc inherit only the guarantees observed at
        # dispatch (g_start).  An instruction's own sem updates fire at
        # write-ack, which is asynchronous wrt the next instruction on the
        # same engine — that's why Tile emits same-engine waits, and we must
        # not treat them as implied by program order.
        proc_g[p] = g_start

    # insert EventSemaphore carriers for the moved waits (per block, back to
    # front so local indices stay valid)
    by_block = {}
    for idx, ins, moved in splits:
        blk, li = inst_loc[idx]
        by_block.setdefault(id(blk), (blk, []))[1].append((li, ins, moved))
    for blk, items in by_block.values():
        new_insts = list(blk.instructions)
        for li, ins, moved in sorted(items, key=lambda x: -x[0]):
            carriers = [
                mybir.InstEventSemaphore(
                    name=nc.get_next_instruction_name(),
                    engine=ins.engine,
                    sync_info=mybir.SyncInfo(on_wait=[w], on_update=[]),
                )
                for w in moved
            ]
            for c in carriers:
                nc.inst_map[c.name] = c
            new_insts[li:li] = carriers
        blk.instructions = new_insts

    return n_pruned


def _sigmoid_f32(v: np.ndarray) -> float:
    # fp32 sigmoid; bit-identical to jax CPU jax.nn.sigmoid for this input
    # (the on-device ACT-table sigmoid is ~36 ULP off — don't use it)
    v32 = np.float32(np.asarray(v).reshape(-1)[0])
    return float(np.float32(1.0) / (np.float32(1.0) + np.exp(-v32, dtype=np.float32)))


def _pack_weights():
    import ml_dtypes

    W = np.zeros((_ROWS, _NG), dtype=ml_dtypes.bfloat16)
    for p in range(_ROWS):
        j = min(p // 24, _NG - 1)
        W[p, j] = ml_dtypes.bfloat16(2.0 ** (p - 24 * j))
    return W


def _decode(outp: np.ndarray, raws: list) -> np.ndarray:
    """outp [72, 4096] packed fp32 ints (t<12), raws 4x [128, 4096] bf16
    (t=12..15) -> spikes [128, 16, 4096] fp32."""
    npk = _T - 4
    V = np.ascontiguousarray(outp.astype(np.uint32)).reshape(npk, _NG, _FD)
    bits = np.unpackbits(
        V.view(np.uint8).reshape(npk, _NG, _FD, 4),
        axis=3,
        bitorder="little",
    )  # [t, j, f, 32]
    S = np.empty((_ROWS, _T, _FD), dtype=np.float32)
    for j in range(_NG):
        nb = 24 if j < _NG - 1 else _ROWS - 24 * (_NG - 1)
        # partitions 24j + i <- bit i of group j
        S[24 * j : 24 * j + nb, :npk, :] = (
            bits[:, j, :, :nb].transpose(2, 0, 1).astype(np.float32)
        )
    for k, r in enumerate(raws):
        # bf16 (t=12/13) or fp8 (t=14/15); spikes are exactly 0/1 in either,
        # so a nonzero-byte test decodes both without dtype dependencies
        a = np.ascontiguousarray(np.asarray(r))
        if a.dtype.itemsize == 1:
            S[:, npk + k, :] = (a.view(np.uint8) != 0).astype(np.float32)
        else:
            S[:, npk + k, :] = a.astype(np.float32)
    return S


def kernel(x: np.ndarray, decay: np.ndarray) -> np.ndarray:
    global last_results
    from concourse.bass_utils import run_bass_kernel_spmd

    x = np.ascontiguousarray(np.asarray(x, dtype=np.float32))
    assert x.shape == (_B, _C, _T, _H, _W), x.shape
    decay_s = _sigmoid_f32(np.asarray(decay, dtype=np.float32))

    nc = _cache.get(decay_s)
    if nc is None:
        nc = _cache[decay_s] = _build(decay_s)

    W = _pack_weights()
    shards = [
        x[i * _BPC : (i + 1) * _BPC].reshape(_ROWS, _T * _FD)
        for i in range(_N_CORES)
    ]
    in_maps = [{"x": s, "w": W} for s in shards]

    tmpdir = os.environ.get("BASS_KERNEL_TMPDIR")
    if tmpdir:
        os.makedirs(tmpdir, exist_ok=True)
    res = run_bass_kernel_spmd(
        nc, in_maps, list(range(_N_CORES)), trace=False, tmpdir=tmpdir
    )
    last_results = res

    out = np.empty((_B, _C, _T, _H, _W), dtype=np.float32)
    for i, r in enumerate(res.results):
        S = _decode(
            r["outp"], [r[f"outs{t}"] for t in range(_T - 4, _T)]
        )  # [128, 16, 4096]
        out[i * _BPC : (i + 1) * _BPC] = S.reshape(_BPC, _C, _T, _H, _W)
    return np.ascontiguousarray(out)
